# revision 11
# baseline (speedup 1.0000x reference)
"""Trainium2 Bass kernel for nn_Attention_v4 (sparse per-atom attention).

Reference computation (fp32):
    x:[2,512,14,1024] -> qkv = x@w_qkv+b_qkv -> per (b, r=atom, head)
    attention over the n=512 axis -> out @ w_proj + b_proj.

Sharding (8 cores): 4 groups x 7 (b,r)-units data-parallel, x 2 head-halves
tensor-parallel. Each core computes, for its 7 units and its 8 heads:
QKV^T projection, attention, and a partial c_proj (contraction over its 512
of the 1024 hd rows). Host unshard sums the two head-half partials and adds
b_proj.

Schedule (all engine/latency choices HW-measured on these cores):
- unit-level software pipeline: stage s emits dma_x(s+2) then interleaves
  [ attn(s) : proj(s+1) + cproj(s-1) ] at ~2 filler matmuls per attention
  item, so the PE always has independent work while the ST->exp->PAV
  chain's ~0.9us/hop semaphore latencies resolve (3 head-pairs of exp->PAV lookahead).
- q/k tiles are bf16 (fast weight load on the PE for the 224 score
  matmuls; drains to bf16 run on DVE at 16-bit rate; scores stay
  well within tolerance at ~1.1e-3 rel err)
- scores contract K=64 directly from the qk tile (387ns vs 547ns for the
  zero-padded K=128 variant; also kills the kz buffer + its DVE copies).
- qk / cproj PSUM->SBUF drains run on ACT as f32 Copy (~0.8us; DVE copies
  are ~1.1us and DVE was the co-bottleneck). exp outputs f32r: ACT bf16
  writes measured 2x slower.
- softmax normalize is fully decoupled from PSUM: raw O^T and the
  denominator reciprocal are pulled out at jt==3 (freeing the PSUM bank
  immediately); the gpsimd broadcast + DVE multiply run deferred,
  SBUF-only, with a whole pipeline stage of slack before cproj reads ot.
  (A naive recip->broadcast->mul chain on the po bank serialized the
  entire attention loop and was worth ~180us/iteration.)
"""

import numpy as np

B, N, A, DIM, H, D = 2, 512, 14, 1024, 16, 64
HL = 8            # heads per core
UNITS = 7         # (b, r) units per group
NCORES = 8
SCALE = np.float32(1.0 / np.sqrt(np.sqrt(D)))
VW = D + 1        # v width per head incl. ones column

_CACHE = {}


def _build_nc(units=UNITS, repeat=1, phases="QAC", qk_bias=False):
    import concourse.bacc as bacc
    import concourse.tile as tile
    from concourse import mybir
    from concourse.bass import ts

    f32, f32r, bf16 = mybir.dt.float32, mybir.dt.float32r, mybir.dt.bfloat16
    AF = mybir.ActivationFunctionType

    nc = bacc.Bacc("TRN2", target_bir_lowering=False, debug=False,
                   num_devices=NCORES)
    xT = nc.dram_tensor("xT", [units, DIM, N], f32r, kind="ExternalInput")
    wqkv = nc.dram_tensor("wqkv", [DIM, 1024 + HL * D], f32r,
                          kind="ExternalInput")
    bqk = nc.dram_tensor("bqk", [1024], f32, kind="ExternalInput")
    bv = nc.dram_tensor("bv", [HL * VW], f32, kind="ExternalInput")
    wproj = nc.dram_tensor("wproj", [HL * D, DIM], f32r, kind="ExternalInput")
    part = nc.dram_tensor("part", [units, N, DIM], f32, kind="ExternalOutput")

    import concourse.bass as bass

    def bcast_part(ap, p=128):
        return bass.AP(tensor=ap.tensor, offset=ap.offset,
                       ap=[[0, p]] + list(ap.ap))

    with tile.TileContext(nc) as tc:
        import contextlib
        with contextlib.ExitStack() as ctx:
            const = ctx.enter_context(tc.tile_pool(name="const", bufs=1))
            p_x = ctx.enter_context(tc.tile_pool(name="p_x", bufs=2))
            p_qk = ctx.enter_context(tc.tile_pool(name="p_qk", bufs=2))
            p_v = ctx.enter_context(tc.tile_pool(name="p_v", bufs=2))
            p_es = ctx.enter_context(tc.tile_pool(name="p_es", bufs=4))
            p_ot = ctx.enter_context(tc.tile_pool(name="p_ot", bufs=2))
            p_out = ctx.enter_context(tc.tile_pool(name="p_out", bufs=2))
            p_rc = ctx.enter_context(tc.tile_pool(name="p_rc", bufs=2))
            p_sc = ctx.enter_context(tc.tile_pool(name="p_sc", bufs=2))
            ps_mm = ctx.enter_context(
                tc.tile_pool(name="ps_mm", bufs=2, space="PSUM"))
            ps_st = ctx.enter_context(
                tc.tile_pool(name="ps_st", bufs=2, space="PSUM"))
            ps_o = ctx.enter_context(
                tc.tile_pool(name="ps_o", bufs=1, space="PSUM"))

            # ---- persistent weights ----
            wq_sb = const.tile([128, 8, 1024 + HL * D], f32r, tag="wqkv")
            _wq_r = wqkv[:].rearrange("(k p) c -> p k c", p=128)
            for k in range(8):
                nc.sync.dma_start(out=wq_sb[:, k, :], in_=_wq_r[:, k, :])
            wp_sb = const.tile([128, 4, DIM], f32r, tag="wproj")
            nc.sync.dma_start(
                out=wp_sb, in_=wproj[:].rearrange("(k p) c -> p k c", p=128))
            bqk_sb = const.tile([128, 8], f32, tag="bqk")
            nc.sync.dma_start(
                out=bqk_sb, in_=bqk[:].rearrange("(c p) -> p c", p=128))
            bv_sb = const.tile([128, HL * VW], f32, tag="bv")
            nc.sync.dma_start(out=bv_sb, in_=bcast_part(bv[:]))

            x_tiles, unit_state, ot_tiles = {}, {}, {}

            def emit_x(u):
                t = p_x.tile([128, 8, N], f32r, tag="x")
                nc.sync.dma_start(
                    out=t, in_=xT[u].rearrange("(k p) n -> p k n", p=128))
                x_tiles[u] = t

            def gen_proj(u):
                x_sb = x_tiles.pop(u)
                qk_sb = p_qk.tile([128, 8, N], bf16, tag="qk")
                v_sb = p_v.tile([128, 4, HL * VW], f32r, tag="v")
                unit_state[u] = (qk_sb, v_sb)

                # qk^T projection: [col, tok]; drains on ACT (bias is
                # per-partition here, so ACT's bias operand applies it free)
                for ct in range(8):
                    pm = ps_mm.tile([128, N], f32, tag="mm")
                    for k in range(8):
                        nc.tensor.matmul(
                            pm, wq_sb[:, k, ts(ct, 128)], x_sb[:, k, :],
                            start=(k == 0), stop=(k == 7))
                        yield
                    with nc.allow_low_precision(reason="bf16 qk scores"):
                        if qk_bias:
                            nc.vector.tensor_scalar_add(
                                qk_sb[:, ct, :], pm, bqk_sb[:, ct:ct + 1])
                        else:
                            nc.vector.tensor_copy(
                                out=qk_sb[:, ct, :], in_=pm)

                # v projection: [tok, lh*65+d]; 65th col per head = 1.0
                vv = v_sb.rearrange("p t (h w) -> p t h w", w=VW)
                bvv = bv_sb.rearrange("p (h w) -> p h w", w=VW)
                for tt in range(4):
                    pv = ps_mm.tile([128, N], f32, tag="mm")
                    pvv = pv.rearrange("p (h d) -> p h d", d=D)
                    for k in range(8):
                        nc.tensor.matmul(
                            pv, x_sb[:, k, ts(tt, 128)],
                            wq_sb[:, k, 1024:1024 + HL * D],
                            start=(k == 0), stop=(k == 7))
                        yield
                    with nc.allow_low_precision(reason="f32r v tile"):
                        nc.vector.tensor_add(
                            out=vv[:, tt, :, 0:D], in0=pvv,
                            in1=bvv[:, :, 0:D])
                        nc.vector.tensor_scalar(
                            out=vv[:, tt, :, D], in0=bvv[:, :, D],
                            scalar1=0.0, scalar2=1.0,
                            op0=mybir.AluOpType.mult,
                            op1=mybir.AluOpType.add)

            def gen_attn(u):
                qk_sb, v_sb = unit_state.pop(u)
                ot_sb = p_ot.tile([128, 4, N], f32r, tag="ot")
                ot_tiles[u] = ot_sb
                pend = []

                def normalize(c, sc, scd):
                    # fully deferred, SBUF-only: raw O^T / den for the head
                    # pair were copied out when the 2-bank po psum tile was
                    # freed; nothing reads ot until cproj in the NEXT pipeline
                    # stage, so this DVE+Pool chain runs off the critical path
                    rc = p_rc.tile([1, 2, N], f32r, tag="rc")
                    bc = p_rc.tile([128, 2, N], f32r, tag="bc")
                    with nc.allow_low_precision(reason="f32r softmax recip"):
                        nc.vector.reciprocal(
                            out=rc.rearrange("p a b -> p (a b)"),
                            in_=scd.rearrange("p a b -> p (a b)"))
                        # broadcast to all 128 partitions (hw broadcast always
                        # starts at partition 0); the muls slice matching rows
                        # (TensorTensor operands must share start partition)
                        nc.gpsimd.partition_broadcast(
                            bc.rearrange("p a b -> p (a b)"),
                            rc.rearrange("p a b -> p (a b)")[0:1, :])
                        nc.vector.tensor_mul(
                            out=ot_sb[0:64, c, :], in0=sc[0:64, :],
                            in1=bc[0:64, 0, :])
                        nc.vector.tensor_mul(
                            out=ot_sb[64:128, c, :], in0=sc[64:128, :],
                            in1=bc[64:128, 1, :])

                # pair-merged: heads (2c, 2c+1) share a 2-bank score psum
                # tile and ONE exp instruction per (c, jt) -> halves the
                # PE->ACT->PE round trips and ACT instruction overhead
                pairs = [(c, jt) for c in range(HL // 2) for jt in range(4)]
                pos, ess = {}, {}

                def gen_st_pair(c, jt):
                    # K=64 scores straight from the qk tile: head lh has its
                    # q cols in qk[:, lh//2] and k cols in qk[:, 4+lh//2],
                    # both on partition rows (lh%2)*64..+64.
                    # The two matmuls sit in disjoint PE row-groups (rows
                    # 0-63 / 64-127 via auto tile_position) and write
                    # different PSUM banks, so issued back-to-back they run
                    # CONCURRENTLY on the array (~512 cycles for the pair,
                    # not 1024). No yield between them: a full-array filler
                    # matmul in the middle would serialize the pair.
                    pst = ps_st.tile([128, 2, N], f32, tag="st")
                    for h01 in range(2):
                        hp = h01 * 64
                        nc.tensor.matmul(
                            pst[:, h01, :],
                            qk_sb[hp:hp + 64, 4 + c, ts(jt, 128)],
                            qk_sb[hp:hp + 64, c, :], start=True, stop=True)
                    yield
                    es_t = p_es.tile([128, 2, N], f32r, tag="es")
                    with nc.allow_low_precision(reason="bf16 softmax"):
                        if "noexp" in phases:  # debug: DVE copy, no ACT
                            nc.vector.tensor_copy(
                                out=es_t.rearrange("p a b -> p (a b)"),
                                in_=pst.rearrange("p a b -> p (a b)"))
                        else:
                            nc.scalar.activation(
                                out=es_t.rearrange("p a b -> p (a b)"),
                                in_=pst.rearrange("p a b -> p (a b)"),
                                func=AF.Exp)
                    ess[(c, jt)] = es_t

                def gen_pav_pair(c, jt):
                    es_t = ess.pop((c, jt))
                    if jt == 0:
                        # one 2-bank psum tile for the whole head pair: the
                        # drain is then a SINGLE DVE copy (rows 0:65 of both
                        # banks -> sbuf scratch), freeing psum ~2 ops + 2 sem
                        # hops sooner than per-head recip+copy chains
                        po_g = ps_o.tile([128, 2, N], f32, tag="o")
                        pos[c] = po_g
                    po = pos[c]
                    for h01 in range(2):
                        lh = 2 * c + h01
                        nc.tensor.matmul(
                            po[0:VW, h01, :],
                            v_sb[:, jt, lh * VW:(lh + 1) * VW],
                            es_t[:, h01, :], start=(jt == 0), stop=(jt == 3))
                        yield
                    if jt == 3:
                        po = pos.pop(c)
                        # head A o -> partitions 0:64, head B o -> 64:128
                        # (Copy may shift partitions; TensorTensor may not),
                        # dens -> a [1, 2, N] tile on partition 0
                        sc = p_sc.tile([128, N], bf16, tag="sc")
                        scd = p_sc.tile([1, 2, N], f32r, tag="scd")
                        with nc.allow_low_precision(reason="bf16 po drain"):
                            nc.vector.tensor_copy(
                                out=sc[0:64, :], in_=po[0:64, 0, :])
                            nc.vector.tensor_copy(
                                out=sc[64:128, :], in_=po[0:64, 1, :])
                            nc.vector.tensor_copy(
                                out=scd.rearrange("p a b -> p (a b)"),
                                in_=po[64:65, :, :].rearrange(
                                    "p a b -> p (a b)"))
                        if "nonorm" not in phases:
                            pend.append((c, sc, scd))
                        if len(pend) > 1:
                            normalize(*pend.pop(0))

                LOOKAHEAD = 3
                for s in range(len(pairs) + LOOKAHEAD):
                    if s < len(pairs):
                        yield from gen_st_pair(*pairs[s])
                    if s >= LOOKAHEAD:
                        yield from gen_pav_pair(*pairs[s - LOOKAHEAD])
                while pend:
                    normalize(*pend.pop(0))

            def gen_cproj(u):
                ot_sb = ot_tiles.pop(u)
                if "C" not in phases:
                    for tt in range(4):
                        nc.sync.dma_start(
                            out=part[u, ts(tt, 128), :],
                            in_=ot_sb[:, 2 * (tt % (ot_sb.shape[1] // 2)):
                                      2 * (tt % (ot_sb.shape[1] // 2)) + 2,
                                      :].bitcast(f32))
                    return
                for tt in range(4):
                    o_sb = p_out.tile([128, DIM], f32, tag="out")
                    for eh in range(2):
                        pc = ps_mm.tile([128, N], f32, tag="mm")
                        for ct in range(4):
                            nc.tensor.matmul(
                                pc, ot_sb[:, ct, ts(tt, 128)],
                                wp_sb[:, ct, eh * 512:(eh + 1) * 512],
                                start=(ct == 0), stop=(ct == 3))
                            yield
                        nc.scalar.activation(
                            out=o_sb[:, eh * 512:(eh + 1) * 512], in_=pc,
                            func=AF.Copy)
                    nc.sync.dma_start(
                        out=part[u, ts(tt, 128), :], in_=o_sb)

            def body():
                from itertools import chain
                emit_x(0)
                if units > 1:
                    emit_x(1)
                for _ in gen_proj(0):
                    pass
                def attn_or_skip(u):
                    if "A" in phases:
                        yield from gen_attn(u)
                    else:
                        qk_sb, v_sb = unit_state.pop(u)
                        ot_tiles[u] = qk_sb
                for s in range(units):
                    if s + 2 < units:
                        emit_x(s + 2)
                    fillers = []
                    nf = 0
                    if s + 1 < units:
                        fillers.append(gen_proj(s + 1))
                        nf += 96
                    if s >= 1:
                        fillers.append(gen_cproj(s - 1))
                        nf += 32
                    filler = chain(*fillers)
                    # spread the filler matmuls evenly over the attention
                    # yields (48 per unit when "A" in phases) so the PE sees
                    # constant pressure instead of 2-per-yield + a tail dump
                    ny = 48 if "A" in phases else 1
                    done = pulled = 0
                    for _ in attn_or_skip(s):
                        done += 1
                        want = (nf * done) // ny
                        while pulled < want:
                            if next(filler, None) is None:
                                pulled = nf
                                break
                            pulled += 1
                    for _ in filler:
                        pass
                for _ in gen_cproj(units - 1):
                    pass

            nbody = 2 if "u2" in phases else 1
            if repeat == 1:
                for _ in range(nbody):
                    body()
            else:
                with tc.For_i(0, repeat, 1):
                    for _ in range(nbody):
                        body()

    nc.compile()
    return nc


def _make_runner(nc, n_cores=NCORES, donate=True):
    """Persistent jitted SPMD runner (mirrors bass2jax.run_bass_via_pjrt)."""
    import jax
    from jax.sharding import Mesh, PartitionSpec
    from jax.experimental.shard_map import shard_map
    from concourse import bass2jax
    from concourse import mybir as mb

    bass2jax.install_neuronx_cc_hook()
    pn = nc.partition_id_tensor.name if nc.partition_id_tensor else None
    in_names, out_names, out_avals, out_shapes = [], [], [], []
    for alloc in nc.m.functions[0].allocations:
        if not isinstance(alloc, mb.MemoryLocationSet):
            continue
        name = alloc.memorylocations[0].name
        if alloc.kind == "ExternalInput":
            if name != pn:
                in_names.append(name)
        elif alloc.kind == "ExternalOutput":
            shape = tuple(alloc.tensor_shape)
            dtype = mb.dt.np(alloc.dtype)
            out_names.append(name)
            out_avals.append(jax.core.ShapedArray(shape, dtype))
            out_shapes.append((shape, dtype))
    n_params = len(in_names)
    n_outs = len(out_names)
    all_in = list(in_names) + list(out_names) + ([pn] if pn else [])

    def _body(*args):
        ops = list(args)
        if pn:
            ops.append(bass2jax.partition_id_tensor())
        return tuple(bass2jax._bass_exec_p.bind(
            *ops, out_avals=tuple(out_avals), in_names=tuple(all_in),
            out_names=tuple(out_names), lowering_input_output_aliases=(),
            sim_require_finite=True, sim_require_nnan=True, nc=nc))

    devices = jax.devices()[:n_cores]
    mesh = Mesh(np.asarray(devices), ("core",))
    specs = (PartitionSpec("core"),)
    fn = jax.jit(
        shard_map(_body, mesh=mesh, in_specs=specs * (n_params + n_outs),
                  out_specs=specs * n_outs, check_rep=False),
        donate_argnums=tuple(range(n_params, n_params + n_outs)) if donate else (),
        keep_unused=True)

    def run(in_maps):
        per_core = [[np.asarray(m[name]) for name in in_names] for m in in_maps]
        concat_in = [np.concatenate([per_core[c][i] for c in range(n_cores)],
                                    axis=0) for i in range(n_params)]
        concat_zeros = [np.zeros((n_cores * s[0], *s[1:]), d)
                        for (s, d) in out_shapes]
        import jax as _jax
        out_arrs = _jax.block_until_ready(fn(*concat_in, *concat_zeros))
        return [
            {name: np.asarray(out_arrs[i]).reshape(n_cores, *out_shapes[i][0])[c]
             for i, name in enumerate(out_names)}
            for c in range(n_cores)
        ]

    run.jit_fn = fn
    run.in_names = in_names
    run.out_names = out_names
    run.out_shapes = out_shapes
    run.n_cores = n_cores
    return run


def _unit_groups():
    units = [(b, r) for b in range(B) for r in range(A)]
    return [units[g * UNITS:(g + 1) * UNITS] for g in range(4)]


def shard_inputs(x, w_qkv, b_qkv, w_proj, b_proj):
    groups = _unit_groups()
    w4 = w_qkv.reshape(DIM, H, 3, D)
    b4 = b_qkv.reshape(H, 3, D)
    in_maps = []
    for c in range(NCORES):
        g, hh = c // 2, c % 2
        heads = list(range(hh * HL, (hh + 1) * HL))
        xT = np.ascontiguousarray(
            np.stack([x[b, :, r, :].T for (b, r) in groups[g]])
        ).astype(np.float32)
        wq = w4[:, heads, 0, :].reshape(DIM, HL * D) * SCALE
        wk = w4[:, heads, 1, :].reshape(DIM, HL * D) * SCALE
        wv = w4[:, heads, 2, :].reshape(DIM, HL * D)
        wqkv_c = np.ascontiguousarray(
            np.concatenate([wq, wk, wv], axis=1)).astype(np.float32)
        bq = (b4[heads, 0, :].reshape(HL * D) * SCALE)
        bk = (b4[heads, 1, :].reshape(HL * D) * SCALE)
        bvv = np.concatenate([b4[heads, 2, :], np.ones((HL, 1), np.float32)],
                             axis=1).reshape(HL * VW)
        in_maps.append({
            "xT": xT,
            "wqkv": wqkv_c,
            "bqk": np.concatenate([bq, bk]).astype(np.float32),
            "bv": bvv.astype(np.float32),
            "wproj": np.ascontiguousarray(
                w_proj[hh * HL * D:(hh + 1) * HL * D, :]).astype(np.float32),
        })
    return in_maps


def unshard(results, b_proj):
    groups = _unit_groups()
    out = np.zeros((B, N, A, DIM), np.float32)
    for g in range(4):
        s = results[2 * g]["part"] + results[2 * g + 1]["part"]
        for idx, (b, r) in enumerate(groups[g]):
            out[b, :, r, :] = s[idx]
    return out + b_proj.astype(np.float32)


def get_runner(qk_bias=False):
    key = ("runner", qk_bias)
    if key not in _CACHE:
        nc = _build_nc(qk_bias=qk_bias)
        _CACHE[key] = _make_runner(nc)
    return _CACHE[key]


def kernel(x, w_qkv, b_qkv, w_proj, b_proj):
    x = np.asarray(x)
    w_qkv = np.asarray(w_qkv)
    b_qkv = np.asarray(b_qkv)
    w_proj = np.asarray(w_proj)
    b_proj = np.asarray(b_proj)
    run = get_runner(qk_bias=bool(np.any(b_qkv[:2048])))
    in_maps = shard_inputs(x, w_qkv, b_qkv, w_proj, b_proj)
    results = run(in_maps)
    return unshard(results, b_proj)



# revision 14
# speedup vs baseline: 1.1081x; 1.1081x over previous
"""Trainium2 Bass kernel for nn_Attention_v4 (sparse per-atom attention).

Reference computation (fp32):
    x:[2,512,14,1024] -> qkv = x@w_qkv+b_qkv -> per (b, r=atom, head)
    attention over the n=512 axis -> out @ w_proj + b_proj.

Sharding (8 cores): 4 groups x 7 (b,r)-units data-parallel, x 2 head-halves
tensor-parallel. Each core computes, for its 7 units and its 8 heads:
QKV^T projection, attention, and a partial c_proj (contraction over its 512
of the 1024 hd rows). Host unshard sums the two head-half partials and adds
b_proj.

Schedule (all engine/latency choices HW-measured on these cores):
- unit-level software pipeline: stage s emits dma_x(s+2) then interleaves
  [ attn(s) : proj(s+1) + cproj(s-1) ] at ~2 filler matmuls per attention
  item, so the PE always has independent work while the ST->exp->PAV
  chain's ~0.9us/hop semaphore latencies resolve (3 head-pairs of exp->PAV lookahead).
- q/k tiles are bf16 (fast weight load on the PE for the 224 score
  matmuls; drains to bf16 run on DVE at 16-bit rate; scores stay
  well within tolerance at ~1.1e-3 rel err)
- scores contract K=64 directly from the qk tile (387ns vs 547ns for the
  zero-padded K=128 variant; also kills the kz buffer + its DVE copies).
- qk / cproj PSUM->SBUF drains run on ACT as f32 Copy (~0.8us; DVE copies
  are ~1.1us and DVE was the co-bottleneck). exp outputs f32r: ACT bf16
  writes measured 2x slower.
- softmax normalize is fully decoupled from PSUM: raw O^T and the
  denominator reciprocal are pulled out at jt==3 (freeing the PSUM bank
  immediately); the gpsimd broadcast + DVE multiply run deferred,
  SBUF-only, with a whole pipeline stage of slack before cproj reads ot.
  (A naive recip->broadcast->mul chain on the po bank serialized the
  entire attention loop and was worth ~180us/iteration.)
"""

import numpy as np

B, N, A, DIM, H, D = 2, 512, 14, 1024, 16, 64
HL = 8            # heads per core
UNITS = 7         # (b, r) units per group
NCORES = 8
SCALE = np.float32(1.0 / np.sqrt(np.sqrt(D)))
VW = D + 1        # v width per head incl. ones column

_CACHE = {}


def _build_nc(units=UNITS, repeat=1, phases="QAC", qk_bias=False):
    import concourse.bacc as bacc
    import concourse.tile as tile
    from concourse import mybir
    from concourse.bass import ts

    f32, f32r, bf16 = mybir.dt.float32, mybir.dt.float32r, mybir.dt.bfloat16
    AF = mybir.ActivationFunctionType

    nc = bacc.Bacc("TRN2", target_bir_lowering=False, debug=False,
                   num_devices=NCORES)
    xT = nc.dram_tensor("xT", [units, DIM, N], f32r, kind="ExternalInput")
    wqkv = nc.dram_tensor("wqkv", [DIM, 1024 + HL * D], f32r,
                          kind="ExternalInput")
    bqk = nc.dram_tensor("bqk", [1024], f32, kind="ExternalInput")
    bv = nc.dram_tensor("bv", [HL * VW], f32, kind="ExternalInput")
    wproj = nc.dram_tensor("wproj", [HL * D, DIM], f32r, kind="ExternalInput")
    part = nc.dram_tensor("part", [units, N, DIM], f32, kind="ExternalOutput")

    import concourse.bass as bass

    def bcast_part(ap, p=128):
        return bass.AP(tensor=ap.tensor, offset=ap.offset,
                       ap=[[0, p]] + list(ap.ap))

    with tile.TileContext(nc) as tc:
        import contextlib
        with contextlib.ExitStack() as ctx:
            const = ctx.enter_context(tc.tile_pool(name="const", bufs=1))
            p_x = ctx.enter_context(tc.tile_pool(name="p_x", bufs=2))
            p_qk = ctx.enter_context(tc.tile_pool(name="p_qk", bufs=2))
            p_v = ctx.enter_context(tc.tile_pool(name="p_v", bufs=2))
            p_es = ctx.enter_context(tc.tile_pool(name="p_es", bufs=5))
            p_ot = ctx.enter_context(tc.tile_pool(name="p_ot", bufs=2))
            p_out = ctx.enter_context(tc.tile_pool(name="p_out", bufs=2))
            p_rc = ctx.enter_context(tc.tile_pool(name="p_rc", bufs=4))
            ps_mm = ctx.enter_context(
                tc.tile_pool(name="ps_mm", bufs=2, space="PSUM"))
            ps_st = ctx.enter_context(
                tc.tile_pool(name="ps_st", bufs=2, space="PSUM"))
            ps_o = ctx.enter_context(
                tc.tile_pool(name="ps_o", bufs=2, space="PSUM"))

            # ---- persistent weights ----
            wq_sb = const.tile([128, 8, 1024 + HL * D], f32r, tag="wqkv")
            _wq_r = wqkv[:].rearrange("(k p) c -> p k c", p=128)
            for k in range(8):
                nc.sync.dma_start(out=wq_sb[:, k, :], in_=_wq_r[:, k, :])
            wp_sb = const.tile([128, 4, DIM], f32r, tag="wproj")
            nc.sync.dma_start(
                out=wp_sb, in_=wproj[:].rearrange("(k p) c -> p k c", p=128))
            bqk_sb = const.tile([128, 8], f32, tag="bqk")
            nc.sync.dma_start(
                out=bqk_sb, in_=bqk[:].rearrange("(c p) -> p c", p=128))
            bv_sb = const.tile([128, HL * VW], f32, tag="bv")
            nc.sync.dma_start(out=bv_sb, in_=bcast_part(bv[:]))

            x_tiles, unit_state, ot_tiles = {}, {}, {}

            def emit_x(u):
                t = p_x.tile([128, 8, N], f32r, tag="x")
                nc.sync.dma_start(
                    out=t, in_=xT[u].rearrange("(k p) n -> p k n", p=128))
                x_tiles[u] = t

            def gen_proj(u):
                x_sb = x_tiles.pop(u)
                qk_sb = p_qk.tile([128, 8, N], bf16, tag="qk")
                v_sb = p_v.tile([128, 4, HL * VW], f32r, tag="v")
                unit_state[u] = (qk_sb, v_sb)

                # qk^T projection: [col, tok]; drains on ACT (bias is
                # per-partition here, so ACT's bias operand applies it free)
                for ct in range(8):
                    pm = ps_mm.tile([128, N], f32, tag="mm")
                    for k in range(8):
                        nc.tensor.matmul(
                            pm, wq_sb[:, k, ts(ct, 128)], x_sb[:, k, :],
                            start=(k == 0), stop=(k == 7))
                        yield
                    with nc.allow_low_precision(reason="bf16 qk scores"):
                        if qk_bias:
                            nc.vector.tensor_scalar_add(
                                qk_sb[:, ct, :], pm, bqk_sb[:, ct:ct + 1])
                        else:
                            nc.vector.tensor_copy(
                                out=qk_sb[:, ct, :], in_=pm)

                # v projection: [tok, lh*65+d]; 65th col per head = 1.0
                vv = v_sb.rearrange("p t (h w) -> p t h w", w=VW)
                bvv = bv_sb.rearrange("p (h w) -> p h w", w=VW)
                for tt in range(4):
                    pv = ps_mm.tile([128, N], f32, tag="mm")
                    pvv = pv.rearrange("p (h d) -> p h d", d=D)
                    for k in range(8):
                        nc.tensor.matmul(
                            pv, x_sb[:, k, ts(tt, 128)],
                            wq_sb[:, k, 1024:1024 + HL * D],
                            start=(k == 0), stop=(k == 7))
                        yield
                    with nc.allow_low_precision(reason="f32r v tile"):
                        nc.vector.tensor_add(
                            out=vv[:, tt, :, 0:D], in0=pvv,
                            in1=bvv[:, :, 0:D])
                        nc.vector.tensor_scalar(
                            out=vv[:, tt, :, D], in0=bvv[:, :, D],
                            scalar1=0.0, scalar2=1.0,
                            op0=mybir.AluOpType.mult,
                            op1=mybir.AluOpType.add)

            def gen_attn(u):
                qk_sb, v_sb = unit_state.pop(u)
                ot_sb = p_ot.tile([128, 4, N], f32r, tag="ot")
                ot_tiles[u] = ot_sb
                pend = []

                def normalize(lh, rc):
                    # fully deferred, SBUF-only: raw O^T was copied to
                    # ot_sb when its psum bank was freed; nothing reads
                    # ot until cproj in the NEXT pipeline stage, so this
                    # Pool+DVE chain runs entirely off the critical path
                    bp = (lh % 2) * 64
                    bc = p_rc.tile([128, N], f32r, tag="bc")
                    # broadcast to all 128 partitions (hw broadcast always
                    # starts at partition 0); the mul slices matching rows
                    nc.gpsimd.partition_broadcast(bc, rc[0:1, :])
                    nc.vector.tensor_mul(
                        out=ot_sb[bp:bp + 64, lh // 2, :],
                        in0=ot_sb[bp:bp + 64, lh // 2, :],
                        in1=bc[bp:bp + 64, :])

                # pair-merged: heads (2c, 2c+1) share a 2-bank score psum
                # tile and ONE exp instruction per (c, jt) -> halves the
                # PE->ACT->PE round trips and ACT instruction overhead
                pairs = [(c, jt) for c in range(HL // 2) for jt in range(4)]
                pos, ess = {}, {}

                def gen_st_pair(c, jt):
                    # K=64 scores straight from the qk tile: head lh has its
                    # q cols in qk[:, lh//2] and k cols in qk[:, 4+lh//2],
                    # both on partition rows (lh%2)*64..+64.
                    # The two matmuls sit in disjoint PE row-groups (rows
                    # 0-63 / 64-127 via auto tile_position) and write
                    # different PSUM banks, so issued back-to-back they run
                    # CONCURRENTLY on the array (~512 cycles for the pair,
                    # not 1024). No yield between them: a full-array filler
                    # matmul in the middle would serialize the pair.
                    pst = ps_st.tile([128, 2, N], f32, tag="st")
                    for h01 in range(2):
                        hp = h01 * 64
                        nc.tensor.matmul(
                            pst[:, h01, :],
                            qk_sb[hp:hp + 64, 4 + c, ts(jt, 128)],
                            qk_sb[hp:hp + 64, c, :], start=True, stop=True)
                    yield
                    es_t = p_es.tile([128, 2, N], f32r, tag="es")
                    with nc.allow_low_precision(reason="bf16 softmax"):
                        if "noexp" in phases:  # debug: DVE copy, no ACT
                            nc.vector.tensor_copy(
                                out=es_t.rearrange("p a b -> p (a b)"),
                                in_=pst.rearrange("p a b -> p (a b)"))
                        else:
                            nc.scalar.activation(
                                out=es_t.rearrange("p a b -> p (a b)"),
                                in_=pst.rearrange("p a b -> p (a b)"),
                                func=AF.Exp)
                    ess[(c, jt)] = es_t

                def gen_pav_pair(c, jt):
                    es_t = ess.pop((c, jt))
                    for h01 in range(2):
                        lh = 2 * c + h01
                        if jt == 0:
                            po = ps_o.tile([128, N], f32, tag="o")
                            pos[lh] = po
                        po = pos[lh]
                        nc.tensor.matmul(
                            po[0:VW, :], v_sb[:, jt, lh * VW:(lh + 1) * VW],
                            es_t[:, h01, :], start=(jt == 0), stop=(jt == 3))
                        yield
                        if jt == 3:
                            po = pos.pop(lh)
                            bp = (lh % 2) * 64
                            rc = p_rc.tile([1, N], f32r, tag="rc")
                            with nc.allow_low_precision(
                                    reason="f32r softmax recip"):
                                nc.vector.reciprocal(
                                    out=rc[0:1, :], in_=po[64:65, :])
                                nc.vector.tensor_copy(
                                    out=ot_sb[bp:bp + 64, lh // 2, :],
                                    in_=po[0:64, :])
                            if "nonorm" not in phases:
                                pend.append((lh, rc))
                            if len(pend) > 3:
                                normalize(*pend.pop(0))

                LOOKAHEAD = 3
                for s in range(len(pairs) + LOOKAHEAD):
                    if s < len(pairs):
                        yield from gen_st_pair(*pairs[s])
                    if s >= LOOKAHEAD:
                        yield from gen_pav_pair(*pairs[s - LOOKAHEAD])
                while pend:
                    normalize(*pend.pop(0))

            def gen_cproj(u):
                ot_sb = ot_tiles.pop(u)
                if "C" not in phases:
                    for tt in range(4):
                        nc.sync.dma_start(
                            out=part[u, ts(tt, 128), :],
                            in_=ot_sb[:, 2 * (tt % (ot_sb.shape[1] // 2)):
                                      2 * (tt % (ot_sb.shape[1] // 2)) + 2,
                                      :].bitcast(f32))
                    return
                for tt in range(4):
                    o_sb = p_out.tile([128, DIM], f32, tag="out")
                    for eh in range(2):
                        pc = ps_mm.tile([128, N], f32, tag="mm")
                        for ct in range(4):
                            nc.tensor.matmul(
                                pc, ot_sb[:, ct, ts(tt, 128)],
                                wp_sb[:, ct, eh * 512:(eh + 1) * 512],
                                start=(ct == 0), stop=(ct == 3))
                            yield
                        nc.scalar.activation(
                            out=o_sb[:, eh * 512:(eh + 1) * 512], in_=pc,
                            func=AF.Copy)
                    nc.sync.dma_start(
                        out=part[u, ts(tt, 128), :], in_=o_sb)

            def body():
                from itertools import chain
                emit_x(0)
                if units > 1:
                    emit_x(1)
                for _ in gen_proj(0):
                    pass
                def attn_or_skip(u):
                    if "A" in phases:
                        yield from gen_attn(u)
                    else:
                        qk_sb, v_sb = unit_state.pop(u)
                        ot_tiles[u] = qk_sb
                for s in range(units):
                    if s + 2 < units:
                        emit_x(s + 2)
                    fillers = []
                    nf = 0
                    if s + 1 < units:
                        fillers.append(gen_proj(s + 1))
                        nf += 96
                    if s >= 1:
                        fillers.append(gen_cproj(s - 1))
                        nf += 32
                    filler = chain(*fillers)
                    # spread the filler matmuls evenly over the attention
                    # yields (48 per unit when "A" in phases) so the PE sees
                    # constant pressure instead of 2-per-yield + a tail dump
                    ny = 48 if "A" in phases else 1
                    done = pulled = 0
                    for _ in attn_or_skip(s):
                        done += 1
                        want = (nf * done) // ny
                        while pulled < want:
                            if next(filler, None) is None:
                                pulled = nf
                                break
                            pulled += 1
                    for _ in filler:
                        pass
                for _ in gen_cproj(units - 1):
                    pass

            nbody = 2 if "u2" in phases else 1
            if repeat == 1:
                for _ in range(nbody):
                    body()
            else:
                with tc.For_i(0, repeat, 1):
                    for _ in range(nbody):
                        body()

    nc.compile()
    return nc


def _make_runner(nc, n_cores=NCORES, donate=True):
    """Persistent jitted SPMD runner (mirrors bass2jax.run_bass_via_pjrt)."""
    import jax
    from jax.sharding import Mesh, PartitionSpec
    from jax.experimental.shard_map import shard_map
    from concourse import bass2jax
    from concourse import mybir as mb

    bass2jax.install_neuronx_cc_hook()
    pn = nc.partition_id_tensor.name if nc.partition_id_tensor else None
    in_names, out_names, out_avals, out_shapes = [], [], [], []
    for alloc in nc.m.functions[0].allocations:
        if not isinstance(alloc, mb.MemoryLocationSet):
            continue
        name = alloc.memorylocations[0].name
        if alloc.kind == "ExternalInput":
            if name != pn:
                in_names.append(name)
        elif alloc.kind == "ExternalOutput":
            shape = tuple(alloc.tensor_shape)
            dtype = mb.dt.np(alloc.dtype)
            out_names.append(name)
            out_avals.append(jax.core.ShapedArray(shape, dtype))
            out_shapes.append((shape, dtype))
    n_params = len(in_names)
    n_outs = len(out_names)
    all_in = list(in_names) + list(out_names) + ([pn] if pn else [])

    def _body(*args):
        ops = list(args)
        if pn:
            ops.append(bass2jax.partition_id_tensor())
        return tuple(bass2jax._bass_exec_p.bind(
            *ops, out_avals=tuple(out_avals), in_names=tuple(all_in),
            out_names=tuple(out_names), lowering_input_output_aliases=(),
            sim_require_finite=True, sim_require_nnan=True, nc=nc))

    devices = jax.devices()[:n_cores]
    mesh = Mesh(np.asarray(devices), ("core",))
    specs = (PartitionSpec("core"),)
    fn = jax.jit(
        shard_map(_body, mesh=mesh, in_specs=specs * (n_params + n_outs),
                  out_specs=specs * n_outs, check_rep=False),
        donate_argnums=tuple(range(n_params, n_params + n_outs)) if donate else (),
        keep_unused=True)

    def run(in_maps):
        per_core = [[np.asarray(m[name]) for name in in_names] for m in in_maps]
        concat_in = [np.concatenate([per_core[c][i] for c in range(n_cores)],
                                    axis=0) for i in range(n_params)]
        concat_zeros = [np.zeros((n_cores * s[0], *s[1:]), d)
                        for (s, d) in out_shapes]
        import jax as _jax
        out_arrs = _jax.block_until_ready(fn(*concat_in, *concat_zeros))
        return [
            {name: np.asarray(out_arrs[i]).reshape(n_cores, *out_shapes[i][0])[c]
             for i, name in enumerate(out_names)}
            for c in range(n_cores)
        ]

    run.jit_fn = fn
    run.in_names = in_names
    run.out_names = out_names
    run.out_shapes = out_shapes
    run.n_cores = n_cores
    return run


def _unit_groups():
    units = [(b, r) for b in range(B) for r in range(A)]
    return [units[g * UNITS:(g + 1) * UNITS] for g in range(4)]


def shard_inputs(x, w_qkv, b_qkv, w_proj, b_proj):
    groups = _unit_groups()
    w4 = w_qkv.reshape(DIM, H, 3, D)
    b4 = b_qkv.reshape(H, 3, D)
    in_maps = []
    for c in range(NCORES):
        g, hh = c // 2, c % 2
        heads = list(range(hh * HL, (hh + 1) * HL))
        xT = np.ascontiguousarray(
            np.stack([x[b, :, r, :].T for (b, r) in groups[g]])
        ).astype(np.float32)
        wq = w4[:, heads, 0, :].reshape(DIM, HL * D) * SCALE
        wk = w4[:, heads, 1, :].reshape(DIM, HL * D) * SCALE
        wv = w4[:, heads, 2, :].reshape(DIM, HL * D)
        wqkv_c = np.ascontiguousarray(
            np.concatenate([wq, wk, wv], axis=1)).astype(np.float32)
        bq = (b4[heads, 0, :].reshape(HL * D) * SCALE)
        bk = (b4[heads, 1, :].reshape(HL * D) * SCALE)
        bvv = np.concatenate([b4[heads, 2, :], np.ones((HL, 1), np.float32)],
                             axis=1).reshape(HL * VW)
        in_maps.append({
            "xT": xT,
            "wqkv": wqkv_c,
            "bqk": np.concatenate([bq, bk]).astype(np.float32),
            "bv": bvv.astype(np.float32),
            "wproj": np.ascontiguousarray(
                w_proj[hh * HL * D:(hh + 1) * HL * D, :]).astype(np.float32),
        })
    return in_maps


def unshard(results, b_proj):
    groups = _unit_groups()
    out = np.zeros((B, N, A, DIM), np.float32)
    for g in range(4):
        s = results[2 * g]["part"] + results[2 * g + 1]["part"]
        for idx, (b, r) in enumerate(groups[g]):
            out[b, :, r, :] = s[idx]
    return out + b_proj.astype(np.float32)


def get_runner(qk_bias=False):
    key = ("runner", qk_bias)
    if key not in _CACHE:
        nc = _build_nc(qk_bias=qk_bias)
        _CACHE[key] = _make_runner(nc)
    return _CACHE[key]


def kernel(x, w_qkv, b_qkv, w_proj, b_proj):
    x = np.asarray(x)
    w_qkv = np.asarray(w_qkv)
    b_qkv = np.asarray(b_qkv)
    w_proj = np.asarray(w_proj)
    b_proj = np.asarray(b_proj)
    run = get_runner(qk_bias=bool(np.any(b_qkv[:2048])))
    in_maps = shard_inputs(x, w_qkv, b_qkv, w_proj, b_proj)
    results = run(in_maps)
    return unshard(results, b_proj)

